# revision 1
# baseline (speedup 1.0000x reference)
"""Multi-head cross attention (B=32, Nq=16384, Nk=31, d_model=64, H=4) on 8 trn2 cores.

Strategy: pure data parallel over batch (4 batches per core). Per batch b the
whole attention is restructured so the only large tensor (Q) is streamed once:

  scores^T[k,q] = (Kblk/sqrt(dh)).T @ Q^T      Kblk: [64, 124] block-diag per head
  E = exp(scores^T + mask_bias)                mask_bias: 0 / -1e30 per k-row
  R = M124.T @ E                               M124: [124,124] block-ones -> per-head row sums
  En = E / R                                   softmax weights (transposed layout)
  out[q,:] = En.T @ VW + b_out                 VW[31h:31h+31,:] = V_h @ W_out[:,hblk].T

Q^T tiles are produced on-chip with PE transposes of naturally-loaded Q tiles.
"""

import os
import sys

for _p in ("/opt/trn_rl_repo", "/opt/pypackages",
           "/root/.axon_site/_ro/trn_rl_repo", "/root/.axon_site/_ro/pypackages"):
    if os.path.isdir(_p) and _p not in sys.path:
        sys.path.insert(0, _p)

import math
import numpy as np

import concourse.bass as bass
import concourse.tile as tile
from concourse import bacc, mybir
from concourse.bass_utils import run_bass_kernel_spmd
from concourse.masks import make_identity

B, NQ, NK, D = 32, 16384, 31, 64
H, DH = 4, 16
SCALE = math.sqrt(DH)
NCORES = 8
BL = B // NCORES          # batches per core
TQ = 512                  # queries per inner iteration
NT = NQ // TQ             # iterations per batch
KB = H * NK               # 124 stacked key rows

MASK_NEG = -1e30

_PROG_CACHE: dict = {}


def _build(mode: str = "f32r", nq: int = NQ):
    """Build the per-core Bass program. mode: 'f32' | 'f32r' for matmul inputs."""
    f32 = mybir.dt.float32
    mmdt = mybir.dt.float32r if mode == "f32r" else f32

    def mmcast(ap):
        return ap

    nt = nq // TQ
    nc = bacc.Bacc("TRN2", target_bir_lowering=False, debug=False, num_devices=NCORES)
    q = nc.dram_tensor("q", [BL * nq, D], f32, kind="ExternalInput").ap()
    kblk = nc.dram_tensor("kblk", [BL, D, KB], mmdt, kind="ExternalInput").ap()
    vw = nc.dram_tensor("vw", [BL, KB, D], mmdt, kind="ExternalInput").ap()
    mbias = nc.dram_tensor("mbias", [BL, KB, 1], f32, kind="ExternalInput").ap()
    m124 = nc.dram_tensor("m124", [KB, KB], mmdt, kind="ExternalInput").ap()
    bias_bc = nc.dram_tensor("bias_bc", [128, (TQ // 128) * D], f32,
                             kind="ExternalInput").ap()
    o = nc.dram_tensor("o", [BL * nq, D], f32, kind="ExternalOutput").ap()

    ntile = TQ // 128  # q-subtiles of 128 per iteration

    with tile.TileContext(nc) as tc:
        with (
            tc.tile_pool(name="singles", bufs=1) as singles,
            tc.tile_pool(name="qin", bufs=3) as qin_pool,
            tc.tile_pool(name="qt_ps", bufs=2, space="PSUM") as qtps_pool,
            tc.tile_pool(name="qt_sb", bufs=2) as qtsb_pool,
            tc.tile_pool(name="st", bufs=2, space="PSUM") as st_pool,
            tc.tile_pool(name="et", bufs=2) as et_pool,
            tc.tile_pool(name="r", bufs=2, space="PSUM") as r_pool,
            tc.tile_pool(name="rinv", bufs=2) as rinv_pool,
            tc.tile_pool(name="en", bufs=2) as en_pool,
            tc.tile_pool(name="u", bufs=2, space="PSUM") as u_pool,
            tc.tile_pool(name="osb", bufs=3) as o_pool,
        ):
            ident = singles.tile([128, 128], f32)
            make_identity(nc, ident)
            kblk_sb = singles.tile([D, BL, KB], mmdt)
            vw_sb = singles.tile([KB, BL, D], mmdt)
            mb_sb = singles.tile([KB, BL], f32)
            m124_sb = singles.tile([KB, KB], mmdt)
            bias_sb = singles.tile([128, ntile * D], f32)
            for b in range(BL):
                nc.sync.dma_start(out=kblk_sb[:, b, :], in_=kblk[b])
                nc.sync.dma_start(out=vw_sb[:, b, :], in_=vw[b])
                nc.sync.dma_start(out=mb_sb[:, b : b + 1], in_=mbias[b])
            nc.sync.dma_start(out=m124_sb, in_=m124)
            nc.sync.dma_start(out=bias_sb, in_=bias_bc)

            for b in range(BL):
                for i in range(nt):
                    row0 = b * nq + i * TQ
                    qin = qin_pool.tile([128, ntile, D], f32)
                    nc.sync.dma_start(
                        out=qin,
                        in_=q[row0 : row0 + TQ].rearrange("(t p) d -> p t d", p=128),
                    )
                    qt_ps = qtps_pool.tile([D, ntile, 128], f32)
                    for t in range(ntile):
                        nc.tensor.transpose(qt_ps[:, t, :], qin[:, t, :], ident)
                    qt_sb = qtsb_pool.tile([D, ntile, 128], mmdt)
                    nc.scalar.copy(qt_sb, qt_ps)

                    st = st_pool.tile([KB, TQ], f32)
                    nc.tensor.matmul(
                        st,
                        mmcast(kblk_sb[:, b, :]),
                        mmcast(qt_sb.rearrange("d t p -> d (t p)")),
                        start=True,
                        stop=True,
                    )
                    et = et_pool.tile([KB, TQ], mmdt)
                    nc.scalar.activation(
                        et, st, mybir.ActivationFunctionType.Exp,
                        bias=mb_sb[:, b : b + 1], scale=1.0,
                    )
                    r = r_pool.tile([KB, TQ], f32)
                    nc.tensor.matmul(r, mmcast(m124_sb), mmcast(et), start=True, stop=True)
                    rinv = rinv_pool.tile([KB, TQ], f32)
                    nc.vector.reciprocal(rinv, r)
                    en = en_pool.tile([KB, TQ], mmdt)
                    nc.vector.tensor_mul(en, et, rinv)

                    u = u_pool.tile([128, ntile, D], f32)
                    for t in range(ntile):
                        nc.tensor.matmul(
                            u[:, t, :],
                            mmcast(en[:, t * 128 : (t + 1) * 128]),
                            mmcast(vw_sb[:, b, :]),
                            start=True,
                            stop=True,
                        )
                    osb = o_pool.tile([128, ntile, D], f32)
                    nc.vector.tensor_add(
                        osb.rearrange("p t d -> p (t d)"),
                        u.rearrange("p t d -> p (t d)"),
                        bias_sb,
                    )
                    nc.sync.dma_start(
                        out=o[row0 : row0 + TQ].rearrange("(t p) d -> p t d", p=128),
                        in_=osb,
                    )
    nc.compile()
    return nc


def _build_bf16(nq: int = NQ):
    """bf16 path. Q arrives host-pre-transposed as qT [BL, 64, nq] so every DMA
    is wide and natural; the output is produced transposed (oT [BL, 64, nq]) and
    un-transposed on the host. Softmax normalization: per-head row-sum matmul ->
    reciprocal_approx_fast -> PE broadcast-expand -> multiply."""
    f32 = mybir.dt.float32
    bf16 = mybir.dt.bfloat16
    nt = nq // TQ

    nc = bacc.Bacc("TRN2", target_bir_lowering=False, debug=False, num_devices=NCORES)
    qT = nc.dram_tensor("qT", [BL, D, nq], bf16, kind="ExternalInput").ap()
    kblk = nc.dram_tensor("kblk", [BL, D, KB], bf16, kind="ExternalInput").ap()
    vw = nc.dram_tensor("vw", [BL, KB, D], bf16, kind="ExternalInput").ap()
    mbias = nc.dram_tensor("mbias", [BL, KB, 1], f32, kind="ExternalInput").ap()
    onesb = nc.dram_tensor("onesb", [KB, H], bf16, kind="ExternalInput").ap()
    exp4 = nc.dram_tensor("exp4", [H, KB], bf16, kind="ExternalInput").ap()
    biasT = nc.dram_tensor("biasT", [D, 1], f32, kind="ExternalInput").ap()
    oT = nc.dram_tensor("oT", [BL, D, nq], f32, kind="ExternalOutput").ap()

    with tile.TileContext(nc) as tc:
        with (
            tc.tile_pool(name="singles", bufs=1) as singles,
            tc.tile_pool(name="qt", bufs=3) as qt_pool,
            tc.tile_pool(name="st", bufs=2, space="PSUM") as st_pool,
            tc.tile_pool(name="et", bufs=2) as et_pool,
            tc.tile_pool(name="r4", bufs=2, space="PSUM") as r4_pool,
            tc.tile_pool(name="rinv", bufs=2) as rinv_pool,
            tc.tile_pool(name="rx", bufs=2, space="PSUM") as rx_pool,
            tc.tile_pool(name="en", bufs=2) as en_pool,
            tc.tile_pool(name="u", bufs=2, space="PSUM") as u_pool,
            tc.tile_pool(name="osb", bufs=3) as o_pool,
        ):
            kblk_sb = singles.tile([D, BL, KB], bf16)
            vw_sb = singles.tile([KB, BL, D], bf16)
            mb_sb = singles.tile([KB, BL], f32)
            onesb_sb = singles.tile([KB, H], bf16)
            exp4_sb = singles.tile([H, KB], bf16)
            biasT_sb = singles.tile([D, 1], f32)
            for b in range(BL):
                nc.sync.dma_start(out=kblk_sb[:, b, :], in_=kblk[b])
                nc.sync.dma_start(out=vw_sb[:, b, :], in_=vw[b])
                nc.sync.dma_start(out=mb_sb[:, b : b + 1], in_=mbias[b])
            nc.sync.dma_start(out=onesb_sb, in_=onesb)
            nc.sync.dma_start(out=exp4_sb, in_=exp4)
            nc.sync.dma_start(out=biasT_sb, in_=biasT)

            for b in range(BL):
                for i in range(nt):
                    col0 = i * TQ
                    qt = qt_pool.tile([D, TQ], bf16)
                    nc.sync.dma_start(out=qt, in_=qT[b, :, col0 : col0 + TQ])

                    st = st_pool.tile([KB, TQ], f32)
                    nc.tensor.matmul(st, kblk_sb[:, b, :], qt, start=True, stop=True)
                    et = et_pool.tile([KB, TQ], bf16)
                    nc.scalar.activation(
                        et, st, mybir.ActivationFunctionType.Exp,
                        bias=mb_sb[:, b : b + 1], scale=1.0,
                    )
                    r4 = r4_pool.tile([H, TQ], f32)
                    nc.tensor.matmul(r4, onesb_sb, et, start=True, stop=True)
                    rinv = rinv_pool.tile([H, TQ], f32)
                    nc.vector.reciprocal_approx_fast(rinv, r4)
                    rinvb = rinv_pool.tile([H, TQ], bf16)
                    nc.vector.tensor_copy(rinvb, rinv)
                    rx = rx_pool.tile([KB, TQ], f32)
                    nc.tensor.matmul(rx, exp4_sb, rinvb, start=True, stop=True)
                    en = en_pool.tile([KB, TQ], bf16)
                    nc.vector.tensor_mul(en, et, rx)

                    u = u_pool.tile([D, TQ], f32)
                    nc.tensor.matmul(u, vw_sb[:, b, :], en, start=True, stop=True)
                    osb = o_pool.tile([D, TQ], f32)
                    nc.vector.tensor_scalar_add(osb, u, biasT_sb)
                    nc.sync.dma_start(out=oT[b, :, col0 : col0 + TQ], in_=osb)
    nc.compile()
    return nc


def _get_program(mode: str):
    if mode not in _PROG_CACHE:
        _PROG_CACHE[mode] = _build_bf16() if mode == "bf16" else _build(mode)
    return _PROG_CACHE[mode]


def _host_prep(Q, K, V, mask, W_out, b_out, mode=None):
    mode = mode or DEFAULT_MODE
    if mode == "bf16":
        import ml_dtypes

        bf = ml_dtypes.bfloat16
        Q = np.asarray(Q, dtype=np.float32)
        K = np.asarray(K, dtype=np.float32)
        V = np.asarray(V, dtype=np.float32)
        W_out = np.asarray(W_out, dtype=np.float32)
        b_out = np.asarray(b_out, dtype=np.float32)
        mask = np.asarray(mask)

        Kblk = np.zeros((B, D, KB), np.float32)
        VW = np.zeros((B, KB, D), np.float32)
        mb = np.zeros((B, KB, 1), np.float32)
        for h in range(H):
            ds, ks = h * DH, h * NK
            Kblk[:, ds : ds + DH, ks : ks + NK] = (
                K[:, :, ds : ds + DH].transpose(0, 2, 1) / SCALE
            )
            VW[:, ks : ks + NK, :] = V[:, :, ds : ds + DH] @ W_out[:, ds : ds + DH].T
            mb[:, ks : ks + NK, 0] = np.where(mask, 0.0, MASK_NEG)
        onesb = np.zeros((KB, H), np.float32)
        for h in range(H):
            onesb[h * NK : (h + 1) * NK, h] = 1.0
        QTb = np.ascontiguousarray(Q.transpose(0, 2, 1)).astype(bf)
        Kblkb = Kblk.astype(bf)
        VWb = VW.astype(bf)
        onesbb = onesb.astype(bf)
        exp4b = onesb.T.copy().astype(bf)
        biasT = b_out[:, None].astype(np.float32)

        in_maps = []
        for c in range(NCORES):
            sl = slice(c * BL, (c + 1) * BL)
            in_maps.append(
                {
                    "qT": QTb[sl],
                    "kblk": Kblkb[sl],
                    "vw": VWb[sl],
                    "mbias": mb[sl],
                    "onesb": onesbb,
                    "exp4": exp4b,
                    "biasT": biasT,
                }
            )
        return in_maps

    Q = np.ascontiguousarray(np.asarray(Q, dtype=np.float32))
    K = np.asarray(K, dtype=np.float32)
    V = np.asarray(V, dtype=np.float32)
    W_out = np.asarray(W_out, dtype=np.float32)
    b_out = np.asarray(b_out, dtype=np.float32)
    mask = np.asarray(mask)

    Kblk = np.zeros((B, D, KB), np.float32)
    VW = np.zeros((B, KB, D), np.float32)
    mb = np.zeros((B, KB, 1), np.float32)
    for h in range(H):
        ds, ks = h * DH, h * NK
        Kblk[:, ds : ds + DH, ks : ks + NK] = (
            K[:, :, ds : ds + DH].transpose(0, 2, 1) / SCALE
        )
        VW[:, ks : ks + NK, :] = V[:, :, ds : ds + DH] @ W_out[:, ds : ds + DH].T
        mb[:, ks : ks + NK, 0] = np.where(mask, 0.0, MASK_NEG)
    M124 = np.zeros((KB, KB), np.float32)
    for h in range(H):
        M124[h * NK : (h + 1) * NK, h * NK : (h + 1) * NK] = 1.0
    bias_bc = np.tile(b_out[None, :], (128, TQ // 128)).astype(np.float32)

    in_maps = []
    for c in range(NCORES):
        sl = slice(c * BL, (c + 1) * BL)
        in_maps.append(
            {
                "q": Q[sl].reshape(BL * NQ, D),
                "kblk": Kblk[sl],
                "vw": VW[sl],
                "mbias": mb[sl],
                "m124": M124,
                "bias_bc": bias_bc,
            }
        )
    return in_maps


def _run(in_maps, mode: str, **kwargs):
    nc = _get_program(mode)
    return run_bass_kernel_spmd(nc, in_maps, list(range(NCORES)), **kwargs)


DEFAULT_MODE = os.environ.get("ATTN_MM_MODE", "f32r")


def kernel(Q, K, V, mask, W_out, b_out):
    in_maps = _host_prep(Q, K, V, mask, W_out, b_out, DEFAULT_MODE)
    res = _run(in_maps, DEFAULT_MODE)
    out = np.empty((B, NQ, D), np.float32)
    for c in range(NCORES):
        if DEFAULT_MODE == "bf16":
            out[c * BL : (c + 1) * BL] = res.results[c]["oT"].transpose(0, 2, 1)
        else:
            out[c * BL : (c + 1) * BL] = res.results[c]["o"].reshape(BL, NQ, D)
    return out



# revision 14
# speedup vs baseline: 2.0433x; 2.0433x over previous
"""Multi-head cross attention (B=32, Nq=16384, Nk=31, d_model=64, H=4) on 8 trn2 cores.

Data parallel over batch (4 per core). Per batch the attention is restructured so
the only large tensor (Q) is streamed once, host-pretransposed to qT [64, Nq] bf16:

  st  = Kblk_aug^T @ qt          Kblk_aug [64,125]: block-diag (K_h/sqrt(dh))^T,
                                 col 124 = 0 (so exp gives a constant ones row)
  et  = exp(st)                  no bias needed: the 0/1 mask is folded
                                 multiplicatively into onesb/VW below
  r4  = onesb^T @ et             onesb [125,5]: masked head-selector; [124,4]=1
                                 -> rows h=0..3 are per-head masked softmax sums,
                                 row 4 = et[124,:] = 1 (normalizer for bias row)
  rinv = 1/r4                    batched: 16 iters of r4 packed into one
                                 [80,512] PSUM bank, one reciprocal_approx_fast
  rx  = PT^T @ rinv5             PT [5,125] head->row broadcast (incl row 124<-4)
  en  = et * rx                  normalized softmax weights (+ ones row)
  u   = VW_aug^T @ en            VW_aug [125,64]: masked V_h @ W_out_h^T blocks,
                                 row 124 = b_out  -> u = out + bias
  out copies pack 2 iters into one [128,512] PSUM bank -> one copy per pair.
"""

import os
import sys

for _p in ("/opt/trn_rl_repo", "/opt/pypackages",
           "/root/.axon_site/_ro/trn_rl_repo", "/root/.axon_site/_ro/pypackages"):
    if os.path.isdir(_p) and _p not in sys.path:
        sys.path.insert(0, _p)

import math
import numpy as np

import concourse.bass as bass
import concourse.tile as tile
from concourse import bacc, mybir
from concourse.bass_utils import run_bass_kernel_spmd

B, NQ, NK, D = 32, 16384, 31, 64
H, DH = 4, 16
SCALE = math.sqrt(DH)
NCORES = 8
BL = B // NCORES          # batches per core
TQ = 512                  # queries per iteration
NT = NQ // TQ             # iterations per batch (32)
GRP = 16                  # iterations per reciprocal batch group
NG = NT // GRP            # groups per batch (2)
KB = H * NK               # 124 stacked key rows
KBA = KB + 1              # +1 ones/bias row
NH5 = H + 1               # 4 heads + bias pseudo-head

QCH = 4096                # q columns per input DMA (8 iters)
OCH = 2048                # q columns per output DMA tile (4 pairs = 8 iters)

_PROG_CACHE: dict = {}


def _build_v2():
    f32 = mybir.dt.float32
    bf16 = mybir.dt.bfloat16

    nc = bacc.Bacc("TRN2", target_bir_lowering=False, debug=False, num_devices=NCORES)
    qT = nc.dram_tensor("qT", [BL, D, NQ], bf16, kind="ExternalInput").ap()
    kblk = nc.dram_tensor("kblk", [BL, D, KBA], bf16, kind="ExternalInput").ap()
    vw = nc.dram_tensor("vw", [BL, KBA, D], bf16, kind="ExternalInput").ap()
    # onesb_pad[b, :, 75-5j : 155-5j] is the [KBA, 80] selector for group
    # iteration j: column 5j+h = masked head-h indicator (h<4), 5j+4 = ones row.
    # All 16 r4 matmuls of a group accumulate into one [80, 512] PSUM tile.
    onesb = nc.dram_tensor("onesb", [BL, KBA, 2 * GRP * NH5], bf16,
                           kind="ExternalInput").ap()
    # ptall[:, j, :] [80, KBA]: row 5j+h_aug(k) = 1 selector for the rx
    # broadcast matmul (rhs = full [80, 512] rinv tile, all base partition 0).
    pt = nc.dram_tensor("pt", [GRP * NH5, GRP, KBA], bf16,
                        kind="ExternalInput").ap()
    # out: per batch, 4 tiles of [128, 2048] bf16 (2-iter row packing x 4 pairs)
    n_otile = NQ // (2 * OCH)      # 4 output tiles per batch (8 iters each)
    opk = nc.dram_tensor("opk", [BL, n_otile, 128, OCH], bf16,
                         kind="ExternalOutput").ap()

    n_pack = GRP * NH5    # 80 rows used of the packed r4 bank

    with tile.TileContext(nc) as tc:
        with (
            tc.tile_pool(name="singles", bufs=1) as singles,
            tc.tile_pool(name="qin", bufs=3) as qin_pool,
            tc.tile_pool(name="st", bufs=2, space="PSUM") as st_pool,
            tc.tile_pool(name="et", bufs=2 * GRP + 4) as et_pool,
            tc.tile_pool(name="r4", bufs=2, space="PSUM") as r4_pool,
            tc.tile_pool(name="rinvf", bufs=2) as rinvf_pool,
            tc.tile_pool(name="rinvb", bufs=2) as rinvb_pool,
            tc.tile_pool(name="rx", bufs=2, space="PSUM") as rx_pool,
            tc.tile_pool(name="en", bufs=4) as en_pool,
            tc.tile_pool(name="u", bufs=2, space="PSUM") as u_pool,
            tc.tile_pool(name="osb", bufs=3) as o_pool,
        ):
            kblk_sb = singles.tile([D, BL, KBA], bf16)
            vw_sb = singles.tile([KBA, BL, D], bf16)
            onesb_sb = singles.tile([KBA, BL, 2 * GRP * NH5], bf16)
            pt_sb = singles.tile([GRP * NH5, GRP, KBA], bf16)
            for b in range(BL):
                nc.sync.dma_start(out=kblk_sb[:, b, :], in_=kblk[b])
                nc.sync.dma_start(out=vw_sb[:, b, :], in_=vw[b])
                nc.sync.dma_start(out=onesb_sb[:, b, :], in_=onesb[b])
            nc.sync.dma_start(out=pt_sb, in_=pt)

            # iteration list: (batch, group, j_within_group)
            iters = [(b, g, j) for b in range(BL) for g in range(NG)
                     for j in range(GRP)]
            ngroups = BL * NG

            # state carried between phases, keyed by group index
            qin_tiles = {}
            et_tiles = {}
            rinvb_tiles = {}
            osb_tiles = {}

            def phase_a(gi, j):
                """st + exp + r4 for iteration j of group gi (pairs: j even does
                DMA bookkeeping)."""
                b, g, _ = iters[gi * GRP]
                it = g * GRP + j          # iteration within batch
                col0 = it * TQ
                if col0 % QCH == 0:
                    qin = qin_pool.tile([D, QCH], bf16, name="qin")
                    nc.sync.dma_start(out=qin, in_=qT[b, :, col0: col0 + QCH])
                    qin_tiles[gi, it // (QCH // TQ)] = qin
                qin = qin_tiles[gi, it // (QCH // TQ)]
                qoff = col0 % QCH

                st = st_pool.tile([KBA, TQ], f32, name="st")
                nc.tensor.matmul(st, kblk_sb[:, b, :], qin[:, qoff: qoff + TQ],
                                 start=True, stop=True)
                et = et_pool.tile([KBA, TQ], bf16, name="et")
                nc.scalar.activation(et, st, mybir.ActivationFunctionType.Exp,
                                     scale=1.0)
                et_tiles[gi, j] = et

                if j == 0:
                    r4_tiles[gi] = r4_pool.tile([n_pack, TQ], f32, name="r4b")
                r4b = r4_tiles[gi]
                c0 = (GRP - 1 - j) * NH5
                nc.tensor.matmul(r4b, onesb_sb[:, b, c0: c0 + n_pack], et,
                                 start=(j == 0), stop=(j == GRP - 1),
                                 skip_group_check=True)

            r4_tiles = {}

            def recip_group(gi):
                r4b = r4_tiles.pop(gi)
                rinvf = rinvf_pool.tile([n_pack, TQ], f32, name="rinvf")
                nc.vector.reciprocal_approx_fast(rinvf, r4b)
                rinvb = rinvb_pool.tile([n_pack, TQ], bf16, name="rinvb")
                nc.vector.tensor_copy(rinvb, rinvf)
                rinvb_tiles[gi] = rinvb

            copy_flip = [0]

            def phase_b(gi, j):
                """rx + mul + u for iteration j of group gi; copy+DMA per pair."""
                b, g, _ = iters[gi * GRP]
                it = g * GRP + j
                rinvb = rinvb_tiles[gi]

                rx = rx_pool.tile([KBA, TQ], f32, name="rx")
                nc.tensor.matmul(rx, pt_sb[:, j, :], rinvb,
                                 start=True, stop=True)
                et = et_tiles.pop((gi, j))
                en = en_pool.tile([KBA, TQ], bf16, name="en")
                nc.vector.tensor_mul(en, et, rx)

                if j % 2 == 0:
                    u_tiles[gi] = u_pool.tile([128, TQ], f32, name="u")
                u = u_tiles[gi]
                nc.tensor.matmul(u[(j % 2) * D:(j % 2) * D + D, :],
                                 vw_sb[:, b, :], en, start=True, stop=True)

                if j % 2 == 1:
                    # one PSUM->SBUF copy per pair, alternating DVE/ACT
                    pairs_per_tile = OCH // TQ            # 4 pairs per osb tile
                    pair = it // 2                        # pair index in batch
                    if pair % pairs_per_tile == 0:
                        osb_tiles[gi] = o_pool.tile([128, OCH], bf16, name="osb")
                    osb = osb_tiles[gi]
                    po = (pair % pairs_per_tile) * TQ
                    if copy_flip[0] % 3 != 2:
                        nc.vector.tensor_copy(osb[:, po: po + TQ], u)
                    else:
                        nc.scalar.copy(osb[:, po: po + TQ], u)
                    copy_flip[0] += 1
                    if pair % pairs_per_tile == pairs_per_tile - 1:
                        nc.sync.dma_start(
                            out=opk[b, it // (2 * pairs_per_tile)], in_=osb)

            u_tiles = {}

            # software pipeline: A(0) fully, then interleave B(g) with A(g+1)
            for j in range(GRP):
                phase_a(0, j)
            recip_group(0)
            for gi in range(ngroups):
                for j in range(GRP):
                    phase_b(gi, j)
                    if gi + 1 < ngroups:
                        phase_a(gi + 1, j)
                if gi + 1 < ngroups:
                    recip_group(gi + 1)

    nc.compile()
    return nc


def _get_program():
    if "v2" not in _PROG_CACHE:
        _PROG_CACHE["v2"] = _build_v2()
    return _PROG_CACHE["v2"]


def _host_prep(Q, K, V, mask, W_out, b_out):
    import ml_dtypes

    bf = ml_dtypes.bfloat16
    Q = np.asarray(Q, dtype=np.float32)
    K = np.asarray(K, dtype=np.float32)
    V = np.asarray(V, dtype=np.float32)
    W_out = np.asarray(W_out, dtype=np.float32)
    b_out = np.asarray(b_out, dtype=np.float32)
    mask = np.asarray(mask)
    m01 = mask.astype(np.float32)                     # [B, NK]

    Kblk = np.zeros((B, D, KBA), np.float32)
    VW = np.zeros((B, KBA, D), np.float32)
    # padded per-group-iteration selector: slice [:, 75-5j : 155-5j] puts the
    # selector block at columns 5j..5j+4 of an [KBA, 80] lhsT
    onesb = np.zeros((B, KBA, 2 * GRP * NH5), np.float32)
    P0 = (GRP - 1) * NH5                  # 75
    # ptall[m, j, k] = 1 iff m == 5j + h_aug(k)
    PTall = np.zeros((GRP * NH5, GRP, KBA), np.float32)
    for h in range(H):
        ds, ks = h * DH, h * NK
        Kblk[:, ds: ds + DH, ks: ks + NK] = (
            K[:, :, ds: ds + DH].transpose(0, 2, 1) / SCALE
        )
        VW[:, ks: ks + NK, :] = (
            (V[:, :, ds: ds + DH] * m01[:, :, None]) @ W_out[:, ds: ds + DH].T
        )
        onesb[:, ks: ks + NK, P0 + h] = m01
        for j in range(GRP):
            PTall[j * NH5 + h, j, ks: ks + NK] = 1.0
    VW[:, KB, :] = b_out[None, :]
    onesb[:, KB, P0 + H] = 1.0
    for j in range(GRP):
        PTall[j * NH5 + H, j, KB] = 1.0

    QT = np.ascontiguousarray(Q.transpose(0, 2, 1)).astype(bf)   # [B, D, NQ]
    Kblkb = Kblk.astype(bf)
    VWb = VW.astype(bf)
    onesbb = onesb.astype(bf)
    PTb = PTall.astype(bf)

    in_maps = []
    for c in range(NCORES):
        sl = slice(c * BL, (c + 1) * BL)
        in_maps.append(
            {
                "qT": QT[sl],
                "kblk": Kblkb[sl],
                "vw": VWb[sl],
                "onesb": onesbb[sl],
                "pt": PTb,
            }
        )
    return in_maps


def _decode_out(res):
    out = np.empty((B, NQ, D), np.float32)
    for c in range(NCORES):
        o = np.asarray(res.results[c]["opk"], dtype=np.float32)
        # o: [BL, 4, 128, 2048] -> [BL, t, half, d, pair, qc]
        o = o.reshape(BL, NQ // (2 * OCH), 2, D, OCH // TQ, TQ)
        # q = ((t*pairs + pair)*2 + half)*TQ + qc
        o = o.transpose(0, 1, 4, 2, 5, 3)     # [BL, t, pair, half, qc, d]
        out[c * BL:(c + 1) * BL] = o.reshape(BL, NQ, D)
    return out


def _run(in_maps, **kwargs):
    nc = _get_program()
    return run_bass_kernel_spmd(nc, in_maps, list(range(NCORES)), **kwargs)


def kernel(Q, K, V, mask, W_out, b_out):
    in_maps = _host_prep(Q, K, V, mask, W_out, b_out)
    res = _run(in_maps)
    return _decode_out(res)
